# revision 4
# baseline (speedup 1.0000x reference)
"""Trainium2 Bass kernel for nn_BaselineGAT (LayerNorm + MLP + GATConv).

Strategy (8 NeuronCores, SPMD, host-mediated phase boundary):
  Phase 1 (per core, nodes sharded 6272/core, degree-bucketed order):
    LayerNorm folded into the first matmul (stats via ones-matmul + Square),
    MLP 1488->1024->512 with bf16 matmuls (fp32 PSUM accumulate), then
    row-major heads. Writes per node: a packed g-table row
    [g (256, c-major) | a_src (8)] in bf16 (768B rows), res (256) f32,
    a_dst (8) f32.
  Host: concat g-table shards -> full table [50176, 384] bf16; patch two
    sentinel rows (g=0, a_src=-200) at rows 0 and 32768 (dummy positions);
    padding gather slots point at the sentinel, so no masking is needed
    (exp(lrelu(-200+a_dst)) ~ 4e-18 and g=0).
  Phase 2 (per core, edges sharded by dst, fused): per 128-dst batch,
    gather src rows in <=32-slot chunks from the two table halves (int16
    gather indices limit a half to 32768 rows), e=lrelu(a_src+a_dst),
    exp written into the row, msg = g*ex in place (c-major layout keeps the
    DVE in 2x 16-bit mode), pairwise-tree reduce [g|.|ex] -> per-batch f32
    accumulator, then normalize by the summed ex, +bg, elu, transpose to
    h-major, +res -> final output rows. No separate merge phase.
"""

import sys

sys.path.insert(0, "/opt/trn_rl_repo")

from dataclasses import dataclass

import numpy as np
import ml_dtypes

import concourse.bass as bass  # noqa: F401
import concourse.mybir as mybir
import concourse.tile as tile
from concourse import bacc
from concourse.bass_utils import run_bass_kernel_spmd
from concourse.library_config import mlp as mlp_lib

P = 128
F32 = mybir.dt.float32
BF16 = mybir.dt.bfloat16
I16 = mybir.dt.int16
AL = mybir.AluOpType
AF = mybir.ActivationFunctionType
NP_BF16 = ml_dtypes.bfloat16


@dataclass
class Cfg:
    n_nodes: int = 50000
    n_edges: int = 800000
    d_in: int = 1488
    d_hid: int = 1024
    d_out: int = 512
    C: int = 32
    H: int = 8
    n_cores: int = 8
    node_chunk: int = 512   # phase-1 nodes per chunk
    split_cap: int = 32768  # max rows addressable by int16 gather idx
    slot_chunk: int = 32    # phase-2 gather slots per compute chunk

    @property
    def d_head(self):  # H*C
        return self.C * self.H

    @property
    def d_in_pad(self):
        return ((self.d_in + P - 1) // P) * P

    @property
    def rows_per_core(self):
        nb = (self.n_nodes + P - 1) // P
        nb = ((nb + self.n_cores - 1) // self.n_cores) * self.n_cores
        return nb // self.n_cores * P

    @property
    def n_batches(self):
        return self.rows_per_core // P

    @property
    def table_rows(self):
        return self.rows_per_core * self.n_cores

    @property
    def split(self):
        return min(self.split_cap, self.table_rows)

    @property
    def row_w(self):
        # packed table row in bf16: [g 256 | a_src 8 | ex-slot 8 | pad],
        # 256B-multiple for dma_gather: 384 elems = 768B
        return 384

    @property
    def tree_w(self):
        # reduced width: [g 256 | junk 8 | ex 8]
        return self.d_head + 2 * self.H


CFG = Cfg()

_NC_CACHE = {}


# ----------------------------------------------------------------------------
# Phase 1: LayerNorm + MLP + heads (bf16)
# ----------------------------------------------------------------------------

def build_phase1(cfg: Cfg):
    key = ("p1", cfg.n_nodes, cfg.node_chunk)
    if key in _NC_CACHE:
        return _NC_CACHE[key]
    nc = bacc.Bacc("TRN2", target_bir_lowering=False)
    R = cfg.rows_per_core
    KT1 = cfg.d_in_pad // P          # k-tiles layer 1 (12)
    KT2 = cfg.d_hid // P             # k-tiles layer 2 (8)
    KT3 = cfg.d_out // P             # k-tiles layer 3 (4)
    MT1 = cfg.d_hid // P             # m-tiles layer 1 (8)
    MT2 = cfg.d_out // P             # m-tiles layer 2 (4)
    NCH = cfg.node_chunk
    chunk_sizes = [NCH] * (R // NCH)
    if R % NCH:
        assert R % NCH % P == 0
        chunk_sizes.append(R % NCH)
    W3 = cfg.d_head + cfg.H          # 264
    DH = cfg.d_head

    xT = nc.dram_tensor("xT", [cfg.d_in_pad, R], BF16, kind="ExternalInput")
    W1p = nc.dram_tensor("W1p", [cfg.d_in_pad, cfg.d_hid], BF16, kind="ExternalInput")
    W2 = nc.dram_tensor("W2", [cfg.d_hid, cfg.d_out], BF16, kind="ExternalInput")
    Wgp = nc.dram_tensor("Wgp", [cfg.d_out, W3], BF16, kind="ExternalInput")
    Wrp = nc.dram_tensor("Wrp", [cfg.d_out, W3], BF16, kind="ExternalInput")
    w1s = nc.dram_tensor("w1s", [8, cfg.d_hid], BF16, kind="ExternalInput")
    onep = nc.dram_tensor("onep", [8, P], BF16, kind="ExternalInput")
    ones1 = nc.dram_tensor("ones1", [P, 1], BF16, kind="ExternalInput")
    cvec = nc.dram_tensor("cvec", [P, MT1], F32, kind="ExternalInput")
    b2v = nc.dram_tensor("b2v", [P, MT2], F32, kind="ExternalInput")
    brpad = nc.dram_tensor("brpad", [P, W3], F32, kind="ExternalInput")

    gtab = nc.dram_tensor("gtab", [R, cfg.row_w], BF16, kind="ExternalOutput")
    res = nc.dram_tensor("res", [R, DH], F32, kind="ExternalOutput")
    adst = nc.dram_tensor("adst", [R, cfg.H], F32, kind="ExternalOutput")

    inv_din = 1.0 / cfg.d_in

    with tile.TileContext(nc) as tc:
        with (
            tc.tile_pool(name="wpool", bufs=1) as wp,
            tc.tile_pool(name="xpool", bufs=2) as xp,
            tc.tile_pool(name="sqpool", bufs=2) as sqp,
            tc.tile_pool(name="hpool", bufs=2) as hp,
            tc.tile_pool(name="epool", bufs=3) as ep,
            tc.tile_pool(name="stat", bufs=1) as stp,
            tc.tile_pool(name="ps_y", bufs=2, space="PSUM") as ps_y,
            tc.tile_pool(name="ps_s", bufs=1, space="PSUM") as ps_s,
            tc.tile_pool(name="ps_o", bufs=1, space="PSUM") as ps_o,
        ):
            w1_sb = wp.tile([P, KT1, cfg.d_hid], BF16)
            nc.sync.dma_start(w1_sb[:], W1p.rearrange("(kt p) m -> p kt m", p=P))
            w2_sb = wp.tile([P, KT2, cfg.d_out], BF16)
            nc.sync.dma_start(w2_sb[:], W2.rearrange("(kt p) m -> p kt m", p=P))
            wg_sb = wp.tile([P, KT3, W3], BF16)
            nc.sync.dma_start(wg_sb[:], Wgp.rearrange("(kt p) m -> p kt m", p=P))
            wr_sb = wp.tile([P, KT3, W3], BF16)
            nc.sync.dma_start(wr_sb[:], Wrp.rearrange("(kt p) m -> p kt m", p=P))
            w1s_sb = wp.tile([8, cfg.d_hid], BF16)
            nc.sync.dma_start(w1s_sb[:], w1s[:])
            onep_sb = wp.tile([8, P], BF16)
            nc.sync.dma_start(onep_sb[:], onep[:])
            ones1_sb = wp.tile([P, 1], BF16)
            nc.sync.dma_start(ones1_sb[:], ones1[:])
            cvec_sb = wp.tile([P, MT1], F32)
            nc.sync.dma_start(cvec_sb[:], cvec[:])
            b2_sb = wp.tile([P, MT2], F32)
            nc.sync.dma_start(b2_sb[:], b2v[:])
            brp_sb = wp.tile([P, W3], F32)
            nc.sync.dma_start(brp_sb[:], brpad[:])

            ns = 0
            for NCH in chunk_sizes:
                # ---- load xT chunk [P, KT1, NCH] (bf16)
                xt = xp.tile([P, KT1, NCH], BF16, tag="xt")
                nc.sync.dma_start(
                    xt[:], xT.rearrange("(kt p) n -> p kt n", p=P)[:, :, ns:ns + NCH]
                )
                # ---- stats: S1 = ones^T @ x ; S2 = ones^T @ x^2
                s1_ps = ps_s.tile([1, NCH], F32, tag="s1")
                s2_ps = ps_s.tile([1, NCH], F32, tag="s2")
                for kt in range(KT1):
                    nc.tensor.matmul(s1_ps[:], ones1_sb[:], xt[:, kt],
                                     start=(kt == 0), stop=(kt == KT1 - 1))
                for kt in range(KT1):
                    xsq = sqp.tile([P, NCH], BF16, tag="xsq")
                    nc.scalar.activation(xsq[:], xt[:, kt], AF.Square)
                    nc.tensor.matmul(s2_ps[:], ones1_sb[:], xsq[:],
                                     start=(kt == 0), stop=(kt == KT1 - 1))
                # ---- finalize stats: mu, rstd
                mu_bf = stp.tile([8, NCH], BF16, tag="mu")
                nc.vector.memset(mu_bf[:], 0.0)
                nc.vector.tensor_scalar_mul(mu_bf[0:1, :], s1_ps[:], inv_din)
                mu_f = stp.tile([1, NCH], F32, tag="muf")
                nc.vector.tensor_scalar_mul(mu_f[:], s1_ps[:], inv_din)
                musq = stp.tile([1, NCH], F32, tag="musq")
                nc.vector.tensor_tensor(musq[:], mu_f[:], mu_f[:], op=AL.mult)
                var = stp.tile([1, NCH], F32, tag="var")
                nc.vector.tensor_scalar(var[:], s2_ps[:], inv_din, None, op0=AL.mult)
                nc.vector.tensor_tensor(var[:], var[:], musq[:], op=AL.subtract)
                nc.vector.tensor_scalar_add(var[:], var[:], 1e-5)
                sd = stp.tile([1, NCH], F32, tag="sd")
                nc.scalar.activation(sd[:], var[:], AF.Sqrt)
                rstd = stp.tile([8, NCH], BF16, tag="rstd")
                nc.vector.memset(rstd[:], 0.0)
                with nc.allow_low_precision(
                        reason="rstd broadcast via bf16 matmul; 0.4% scale ok"):
                    nc.vector.reciprocal(rstd[0:1, :], sd[:])
                # broadcast rstd to [P, NCH] via K=8 matmul
                rb_ps = ps_s.tile([P, NCH], F32, tag="rb")
                nc.tensor.matmul(rb_ps[:], onep_sb[:], rstd[:], start=True, stop=True)
                rstd_b = stp.tile([P, NCH], F32, tag="rstdb")
                nc.vector.tensor_copy(rstd_b[:], rb_ps[:])

                # ---- layer 1: y = W1p^T x - w1sum (x) mu ; h = relu(y*rstd + c)
                h_sb = hp.tile([P, MT1, NCH], BF16, tag="h")
                for mt in range(MT1):
                    y_ps = ps_y.tile([P, NCH], F32, tag="y")
                    for kt in range(KT1):
                        nc.tensor.matmul(y_ps[:], w1_sb[:, kt, mt * P:(mt + 1) * P],
                                         xt[:, kt], start=(kt == 0), stop=False)
                    nc.tensor.matmul(y_ps[:], w1s_sb[:, mt * P:(mt + 1) * P], mu_bf[:],
                                     start=False, stop=True)
                    tmp = ep.tile([P, NCH], F32, tag="l1t")
                    nc.vector.tensor_tensor(tmp[:], y_ps[:], rstd_b[:], op=AL.mult)
                    nc.scalar.activation(h_sb[:, mt], tmp[:], AF.Relu,
                                         bias=cvec_sb[:, mt:mt + 1])

                # ---- layer 2: h2 = W2^T h + b2
                h2_sb = hp.tile([P, MT2, NCH], BF16, tag="h2")
                for mt in range(MT2):
                    y2_ps = ps_y.tile([P, NCH], F32, tag="y")
                    for kt in range(KT2):
                        nc.tensor.matmul(y2_ps[:], w2_sb[:, kt, mt * P:(mt + 1) * P],
                                         h_sb[:, kt], start=(kt == 0), stop=(kt == KT2 - 1))
                    nc.scalar.activation(h2_sb[:, mt], y2_ps[:], AF.Identity,
                                         bias=b2_sb[:, mt:mt + 1])

                # ---- layer 3 (row-major): per 128-node subtile
                for nt in range(NCH // P):
                    g_ps = ps_o.tile([P, W3], F32, tag="gps")
                    r_ps = ps_o.tile([P, W3], F32, tag="rps")
                    for kt in range(KT3):
                        nc.tensor.matmul(g_ps[:], h2_sb[:, kt, nt * P:(nt + 1) * P],
                                         wg_sb[:, kt], start=(kt == 0), stop=(kt == KT3 - 1))
                    for kt in range(KT3):
                        nc.tensor.matmul(r_ps[:], h2_sb[:, kt, nt * P:(nt + 1) * P],
                                         wr_sb[:, kt], start=(kt == 0), stop=(kt == KT3 - 1))
                    gt = ep.tile([P, W3], BF16, tag="gt")
                    nc.vector.tensor_copy(gt[:], g_ps[:])
                    rt = ep.tile([P, W3], F32, tag="rt")
                    nc.vector.tensor_tensor(rt[:], r_ps[:], brp_sb[:], op=AL.add)
                    r0 = ns + nt * P
                    nc.sync.dma_start(gtab[r0:r0 + P, :W3], gt[:])
                    nc.sync.dma_start(res[r0:r0 + P, :], rt[:, :DH])
                    nc.sync.dma_start(adst[r0:r0 + P, :], rt[:, DH:W3])
                ns += NCH
    nc.compile()
    _NC_CACHE[key] = nc
    return nc


# ----------------------------------------------------------------------------
# Phase 2: fused edge pass + epilogue
# ----------------------------------------------------------------------------

def build_phase2(cfg: Cfg, Ka: list, Kb: list):
    """Ka/Kb: per-batch slot capacities for the A half (table[:split]) and
    B half (table[split:]). Joint layout per batch: [A slots | B slots]."""
    key = ("p2", cfg.n_nodes, tuple(Ka), tuple(Kb))
    if key in _NC_CACHE:
        return _NC_CACHE[key]
    nc = bacc.Bacc("TRN2", target_bir_lowering=False)
    R = cfg.rows_per_core
    NB = cfg.n_batches
    RW = cfg.row_w
    TW = cfg.tree_w        # 272
    DH = cfg.d_head        # 256
    H = cfg.H
    C = cfg.C
    SC = cfg.slot_chunk    # 32
    assert len(Ka) == NB and len(Kb) == NB
    cols = 8 * (sum(Ka) + sum(Kb))

    gtab = nc.dram_tensor("gtab", [cfg.table_rows, RW], BF16, kind="ExternalInput")
    idx = nc.dram_tensor("idx", [P, cols], I16, kind="ExternalInput")
    adt = nc.dram_tensor("adt", [P, NB, H], BF16, kind="ExternalInput")
    resi = nc.dram_tensor("resi", [R, DH], F32, kind="ExternalInput")
    bgb = nc.dram_tensor("bgb", [P, DH], F32, kind="ExternalInput")
    outp = nc.dram_tensor("outp", [R, DH], F32, kind="ExternalOutput")

    with tile.TileContext(nc) as tc:
        with (
            tc.tile_pool(name="const", bufs=1) as cp,
            tc.tile_pool(name="gath", bufs=3) as gp,
            tc.tile_pool(name="wk", bufs=3) as wk,
            tc.tile_pool(name="accp", bufs=2) as accp,
            tc.tile_pool(name="resp", bufs=2) as rp,
            tc.tile_pool(name="outp_", bufs=2) as op_,
        ):
            nc.gpsimd.load_library(mlp_lib)
            idx_sb = cp.tile([P, cols], I16)
            nc.sync.dma_start(idx_sb[:], idx[:])
            adt_sb = cp.tile([P, NB, H], BF16)
            nc.sync.dma_start(adt_sb[:], adt[:])
            bg_sb = cp.tile([P, DH], F32)
            nc.sync.dma_start(bg_sb[:], bgb[:])

            tabA = gtab[:cfg.split, :]
            tabB = gtab[cfg.split:, :]

            off = 0  # global slot offset into idx
            for b in range(NB):
                acc = accp.tile([P, TW], F32, tag="acc", name=f"acc{b}")
                res_t = rp.tile([P, DH], F32, tag="res", name=f"res{b}")
                nc.sync.dma_start(
                    res_t[:], resi.rearrange("(b p) w -> p b w", p=P)[:, b])
                first = True
                for tab_ap, Kh in ((tabA, Ka[b]), (tabB, Kb[b])):
                    for c0 in range(0, Kh, SC):
                        kc = min(SC, Kh - c0)
                        gt_full = gp.tile([P, SC, RW], BF16, tag="gt",
                                          name=f"g{b}_{off}")
                        gt = gt_full[:, :kc, :]
                        for k0 in range(0, kc, 8):
                            kk = min(8, kc - k0)
                            ni = P * kk
                            nc.gpsimd.dma_gather(
                                gt[:, k0:k0 + kk, :], tab_ap,
                                idx_sb[:, 8 * (off + k0):8 * (off + k0 + kk)],
                                ni, ni, RW,
                            )
                        # e = lrelu(a_src + a_dst); ex = exp(e) -> row slot
                        e_t = wk.tile([P, SC, H], BF16, tag="et")
                        nc.vector.tensor_tensor(
                            e_t[:, :kc], gt[:, :, DH:DH + H],
                            adt_sb[:, b, :].unsqueeze(1).to_broadcast([P, kc, H]),
                            op=AL.add)
                        nc.vector.scalar_tensor_tensor(
                            e_t[:, :kc], e_t[:, :kc], 0.2, e_t[:, :kc],
                            op0=AL.mult, op1=AL.max)
                        nc.scalar.activation(gt[:, :, DH + H:TW], e_t[:, :kc], AF.Exp)
                        # msg = g * ex (broadcast ex over C; c-major keeps 2x)
                        nc.vector.tensor_tensor(
                            gt[:, :, :DH].rearrange("p k (c h) -> p k c h", h=H),
                            gt[:, :, :DH].rearrange("p k (c h) -> p k c h", h=H),
                            gt[:, :, DH + H:TW].unsqueeze(2).to_broadcast(
                                [P, kc, C, H]),
                            op=AL.mult)
                        # pairwise-tree reduce over slots (bf16, packed rows)
                        k = kc
                        while k > 1:
                            hh = (k + 1) // 2
                            lo = k - hh
                            nc.vector.tensor_tensor(
                                gt[:, :lo, :TW], gt[:, :lo, :TW],
                                gt[:, hh:k, :TW], op=AL.add)
                            k = hh
                        if first:
                            nc.vector.tensor_copy(acc[:], gt[:, 0, :TW])
                            first = False
                        else:
                            nc.vector.tensor_tensor(acc[:], acc[:], gt[:, 0, :TW],
                                                    op=AL.add)
                        off += kc
                # ---- epilogue for batch b
                rec = wk.tile([P, H], F32, tag="rec")
                nc.vector.reciprocal(rec[:], acc[:, DH + H:TW])
                nc.vector.tensor_tensor(
                    acc[:, :DH].rearrange("p (c h) -> p c h", h=H),
                    acc[:, :DH].rearrange("p (c h) -> p c h", h=H),
                    rec[:].unsqueeze(1).to_broadcast([P, C, H]),
                    op=AL.mult)
                nc.vector.tensor_tensor(acc[:, :DH], acc[:, :DH], bg_sb[:],
                                        op=AL.add)
                zm = wk.tile([P, DH], F32, tag="zm")
                nc.vector.tensor_scalar_min(zm[:], acc[:, :DH], 0.0)
                ez = wk.tile([P, DH], F32, tag="ez")
                nc.scalar.activation(ez[:], zm[:], AF.Exp)
                o_cm = op_.tile([P, DH], F32, tag="ocm")
                nc.vector.scalar_tensor_tensor(o_cm[:], acc[:, :DH], 0.0, ez[:],
                                               op0=AL.max, op1=AL.add)
                # transpose c-major -> h-major, -1, +res in one op
                o_hm = op_.tile([P, DH], F32, tag="ohm")
                nc.vector.scalar_tensor_tensor(
                    o_hm[:].rearrange("p (h c) -> p h c", c=C),
                    o_cm[:].rearrange("p (c h) -> p c h", h=H).transpose([0, 2, 1]),
                    -1.0,
                    res_t[:].rearrange("p (h c) -> p h c", c=C),
                    op0=AL.add, op1=AL.add)
                nc.sync.dma_start(
                    outp.rearrange("(b p) w -> p b w", p=P)[:, b], o_hm[:])
    nc.compile()
    _NC_CACHE[key] = nc
    return nc


# ----------------------------------------------------------------------------
# Host-side preparation
# ----------------------------------------------------------------------------

def wrap_idx(lst: np.ndarray) -> np.ndarray:
    """list index i -> sbuf [16-wrap x 8 replication]: [p, col] = lst[col*16 + p%16]."""
    n = len(lst)
    assert n % 16 == 0
    lay = lst.reshape(n // 16, 16).T.copy()
    return np.tile(lay, (8, 1)).astype(np.int16)


def prep(cfg: Cfg, x, edge_index, ln_g, ln_b, W1, b1, W2, b2, Wr, br, Wg,
         att_src, att_dst, bg):
    """Everything host-side: sharding, permutations, idx arrays, weight prep."""
    N = cfg.n_nodes
    R = cfg.rows_per_core
    NB = cfg.n_batches
    NCORE = cfg.n_cores
    TR = cfg.table_rows
    H, C = cfg.H, cfg.C

    x = np.asarray(x, np.float32)
    ln_g = np.asarray(ln_g, np.float32)
    ln_b = np.asarray(ln_b, np.float32)
    W1 = np.asarray(W1, np.float32)
    b1 = np.asarray(b1, np.float32)
    W2 = np.asarray(W2, np.float32)
    b2 = np.asarray(b2, np.float32)
    Wr = np.asarray(Wr, np.float32)
    br = np.asarray(br, np.float32)
    Wg = np.asarray(Wg, np.float32)
    att_src = np.asarray(att_src, np.float32)
    att_dst = np.asarray(att_dst, np.float32)
    bg = np.asarray(bg, np.float32)

    src = np.asarray(edge_index[0], np.int64)
    dst = np.asarray(edge_index[1], np.int64)
    loops = np.arange(N, dtype=np.int64)
    src = np.concatenate([src, loops])
    dst = np.concatenate([dst, loops])

    deg = np.bincount(dst, minlength=N)  # in-degree incl self loop

    # ---- provisional node -> position: degree-sorted blocks, round-robin
    order0 = np.argsort(deg, kind="stable")
    padded = np.full(TR, -1, np.int64)
    padded[:N] = order0
    blocks = padded.reshape(TR // P, P)
    core_nodes0 = [[] for _ in range(NCORE)]
    for j in range(blocks.shape[0]):
        core_nodes0[j % NCORE].append(blocks[j])
    core_nodes0 = [np.concatenate(bl) for bl in core_nodes0]

    pos0 = np.full(N, -1, np.int64)
    for c in range(NCORE):
        ids = core_nodes0[c]
        msk = ids >= 0
        pos0[ids[msk]] = c * R + np.nonzero(msk)[0]

    srcA0 = pos0[src] < cfg.split
    degA0 = np.bincount(dst[srcA0], minlength=N)
    degB0 = deg - degA0

    # final assignment: sort (padded) nodes by provisional (degA, degB) so
    # every core's batch b covers the same degA/degB range, re-deal blocks
    keyA = np.where(padded >= 0,
                    degA0[np.maximum(padded, 0)] * 4096
                    + degB0[np.maximum(padded, 0)], -1)
    gorder = np.argsort(keyA, kind="stable")
    sorted_nodes = padded[gorder]

    # force dummies (-1) to global positions 0 and split (sentinel rows).
    # dummies currently sit wherever keyA == -1 sorted them (the front).
    dummy_pos = np.nonzero(sorted_nodes < 0)[0]
    assert len(dummy_pos) >= 2, "need >=2 dummy rows for sentinels"
    # the deal maps sorted-global-index G -> core (G//P) % NCORE,
    # batch (G//P)//NCORE, partition G % P; global table position:
    # core*R + batch*P + partition.
    def table_pos_of_sorted(Gi):
        blk = Gi // P
        return (blk % NCORE) * R + (blk // NCORE) * P + (Gi % P)

    # want table positions 0 and split occupied by dummies: find the sorted
    # indices that land there and swap dummies in.
    targets = [0, cfg.split]
    tp = table_pos_of_sorted(np.arange(TR))
    for t in targets:
        gi = int(np.nonzero(tp == t)[0][0])
        if sorted_nodes[gi] >= 0:
            dj = int(dummy_pos[0]) if sorted_nodes[int(dummy_pos[0])] < 0 else None
            # find a dummy position not already used at a target
            for dcand in dummy_pos:
                gj = int(dcand)
                if sorted_nodes[gj] < 0 and tp[gj] not in targets:
                    sorted_nodes[gi], sorted_nodes[gj] = (
                        sorted_nodes[gj], sorted_nodes[gi])
                    break
            else:
                raise RuntimeError("no free dummy for sentinel swap")

    blocks2 = sorted_nodes.reshape(TR // P, P)
    core_nodes = [[] for _ in range(NCORE)]
    for j in range(blocks2.shape[0]):
        core_nodes[j % NCORE].append(blocks2[j])
    core_nodes = [np.concatenate(bl) for bl in core_nodes]
    pos = np.full(N, -1, np.int64)
    for c in range(NCORE):
        ids = core_nodes[c]
        msk = ids >= 0
        pos[ids[msk]] = c * R + np.nonzero(msk)[0]
    assert core_nodes[0][0] < 0 and core_nodes[cfg.split // R][cfg.split % R] < 0

    # exact halves under final pos
    spos = pos[src]
    dpos = pos[dst]
    isA = spos < cfg.split
    degA = np.zeros(TR, np.int64)
    np.add.at(degA, dpos[isA], 1)
    degB = np.zeros(TR, np.int64)
    np.add.at(degB, dpos[~isA], 1)

    # shared batch capacities (max over cores)
    degA_m = degA.reshape(NCORE, NB, P)
    Ka = np.maximum(1, degA_m.max(axis=(0, 2))).astype(np.int64)
    degB_m = degB.reshape(NCORE, NB, P)
    Kb = np.maximum(1, degB_m.max(axis=(0, 2))).astype(np.int64)

    # ---- per-core edge slot assignment + idx arrays (joint [A|B] layout)
    core = dpos // R
    row = dpos % R
    soff = np.concatenate([[0], np.cumsum(Ka + Kb)])  # slot offset per batch
    nslots = int(soff[-1])
    lists = [np.zeros(nslots * P, np.int64) for _ in range(NCORE)]

    def fill(sel, base_in_batch, base_tab):
        sp = spos[sel] - base_tab
        cr = core[sel]
        rw = row[sel]
        b = rw // P
        p = rw % P
        key = cr * R + rw
        srt = np.argsort(key, kind="stable")
        kk = key[srt]
        grp_start = np.r_[0, np.nonzero(np.diff(kk))[0] + 1]
        sizes = np.diff(np.r_[grp_start, len(kk)])
        within = np.arange(len(kk)) - np.repeat(grp_start, sizes)
        ks = np.zeros(sel.sum(), np.int64)
        ks[srt] = within
        li = (soff[b] + base_in_batch[b] + ks) * P + p
        for c in range(NCORE):
            m = cr == c
            lists[c][li[m]] = sp[m]

    zero_base = np.zeros(NB, np.int64)
    fill(isA, zero_base, 0)
    fill(~isA, Ka, cfg.split)

    idx_w = [wrap_idx(lists[c]) for c in range(NCORE)]

    # ---- phase-1 inputs
    W1p = W1 * ln_g[:, None]
    W1pad = np.zeros((cfg.d_in_pad, cfg.d_hid), np.float32)
    W1pad[:cfg.d_in] = W1p
    w1s = np.zeros((8, cfg.d_hid), np.float32)
    w1s[0] = -W1pad.sum(axis=0)
    cvec_flat = b1 + ln_b @ W1
    cvec = cvec_flat.reshape(cfg.d_hid // P, P).T.astype(np.float32).copy()
    b2t = b2.reshape(cfg.d_out // P, P).T.astype(np.float32).copy()
    onep = np.zeros((8, P), np.float32)
    onep[0] = 1.0
    ones1 = np.ones((P, 1), np.float32)

    att_src_e = np.zeros((cfg.d_head, H), np.float32)
    att_dst_e = np.zeros((cfg.d_head, H), np.float32)
    for h in range(H):
        att_src_e[h * C:(h + 1) * C, h] = att_src[h]
        att_dst_e[h * C:(h + 1) * C, h] = att_dst[h]
    # c-major column permutation for the g table: col c*H+h <- h*C+c
    cm_perm = (np.arange(cfg.d_head).reshape(C, H).T.flatten())  # maps? see below
    # We want Wg_cm[:, c*H + h] = Wg[:, h*C + c]:
    cm_cols = np.empty(cfg.d_head, np.int64)
    for c in range(C):
        for h in range(H):
            cm_cols[c * H + h] = h * C + c
    Wg_cm = Wg[:, cm_cols]
    Wgp = np.concatenate([Wg_cm, Wg @ att_src_e], axis=1).astype(np.float32)
    Wrp = np.concatenate([Wr + 0.0, Wg @ att_dst_e], axis=1).astype(np.float32)

    xts = []
    for c in range(NCORE):
        ids = core_nodes[c]
        xs = np.zeros((R, cfg.d_in), np.float32)
        msk = ids >= 0
        xs[msk] = x[ids[msk]]
        xt = np.zeros((cfg.d_in_pad, R), np.float32)
        xt[:cfg.d_in] = xs.T
        xts.append(xt.astype(NP_BF16))

    bg_cm = bg.reshape(H, C).T.flatten().astype(np.float32)
    bg_b = np.tile(bg_cm, (P, 1))
    W3 = cfg.d_head + cfg.H
    brpad_t = np.zeros((P, W3), np.float32)
    brpad_t[:, :cfg.d_head] = np.tile(br.astype(np.float32), (P, 1))

    meta = dict(core_nodes=core_nodes, pos=pos, Ka=Ka, Kb=Kb,
                idx=idx_w, bg_b=bg_b)
    p1_shared = dict(
        W1p=W1pad.astype(NP_BF16), W2=W2.astype(NP_BF16),
        Wgp=Wgp.astype(NP_BF16), Wrp=Wrp.astype(NP_BF16),
        w1s=w1s.astype(NP_BF16), onep=onep.astype(NP_BF16),
        ones1=ones1.astype(NP_BF16), cvec=cvec, b2v=b2t, brpad=brpad_t)
    p1_maps = [dict(xT=xts[c], **p1_shared) for c in range(NCORE)]
    return p1_maps, meta


def make_sentinel_row(cfg: Cfg) -> np.ndarray:
    row = np.zeros(cfg.row_w, NP_BF16)
    row[cfg.d_head:cfg.d_head + cfg.H] = NP_BF16(-200.0)
    return row


def build_p2_maps(cfg: Cfg, meta, gtabs, ress, adsts):
    gtab_full = np.concatenate(gtabs, axis=0)  # [TR, 384] bf16
    sent = make_sentinel_row(cfg)
    gtab_full[0] = sent
    gtab_full[cfg.split] = sent
    p2_maps = []
    for c in range(cfg.n_cores):
        ad = adsts[c]  # [R, H] f32, π1 order
        adt = ad.reshape(cfg.n_batches, P, cfg.H).transpose(1, 0, 2)
        p2_maps.append(dict(
            gtab=gtab_full, idx=meta["idx"][c],
            adt=adt.astype(NP_BF16).copy(),
            resi=ress[c], bgb=meta["bg_b"],
        ))
    return p2_maps


def kernel(**inputs) -> np.ndarray:
    cfg = CFG
    N = cfg.n_nodes
    NCORE = cfg.n_cores
    DH = cfg.d_head

    p1_maps, meta = prep(cfg, **inputs)

    nc1 = build_phase1(cfg)
    r1 = run_bass_kernel_spmd(nc1, p1_maps, core_ids=list(range(NCORE)))
    gtabs = [r1.results[c]["gtab"] for c in range(NCORE)]
    ress = [r1.results[c]["res"] for c in range(NCORE)]
    adsts = [r1.results[c]["adst"] for c in range(NCORE)]

    Ka = [int(k) for k in meta["Ka"]]
    Kb = [int(k) for k in meta["Kb"]]
    nc2 = build_phase2(cfg, Ka, Kb)
    p2_maps = build_p2_maps(cfg, meta, gtabs, ress, adsts)
    r2 = run_bass_kernel_spmd(nc2, p2_maps, core_ids=list(range(NCORE)))

    out = np.zeros((N, DH), np.float32)
    for c in range(NCORE):
        ids = meta["core_nodes"][c]
        msk = ids >= 0
        out[ids[msk]] = r2.results[c]["outp"][msk]
    return out
